# revision 6
# baseline (speedup 1.0000x reference)
"""Trainium2 Bass kernel for the separable transpose-conv (wavelet synthesis) layer.

Full op: x [16, 128, 128, 144] f32 -> out [16, 256, 256, 16] f32.
Two passes of grouped 1D transpose convs (stride 2, 9 taps, 3ch->1ch) with
symmetric padding + border multipliers, separable over W then H.

Formulation: each pass folds (symmetric pad + border multiplier + polyphase
transpose conv + crop) into a constant banded matrix A[cc] of shape [128, 256]
per within-triplet channel cc (columns 0:128 = even outputs, 128:256 = odd).

  pass 1 (W):  z[b,h,g,v]   = sum_w sum_cc x[b,h,w,3g+cc] * A[cc][w,v]
  pass 2 (H):  o[b,m,G2,v]  = sum_h sum_gg A[gg][h,m] * z[b,h,3G2+gg,v]

Both passes map onto PE matmuls with the spatial conv axis as the contraction
(partition) dim; the 3-way channel mixing becomes 3 PSUM-accumulated matmuls.
H == W == 128 so the same A matrices serve both passes.

Sharding: pure data parallel, batch 16 -> 2 per core across 8 cores (SPMD).
"""

import numpy as np

N_CORES = 8
B_FULL = 16
B_PER = B_FULL // N_CORES  # 2
H = 128
W = 128
C = 144
G = C // 3    # 48
G2 = C // 9   # 16

# Matmul dtype: bf16 streams 1 row/cycle with hidden weight loads (measured
# 109ns/MM @ N=256); fp32/fp32r pay a 2-4x weight-load/row penalty. Inputs are
# cast to bf16 host-side, which also halves the HBM load traffic.
_USE_BF16 = True


def _build_A():
    """A [3, 128, 256] f32: banded matrices with pad reflection + border
    multiplier folded in. Validated against the jax reference to ~1e-7 rel."""
    t = np.arange(27, dtype=np.float64).reshape(3, 9)
    inv = (np.cos(t * np.float32(0.7)).astype(np.float32) * 0.5).astype(np.float32)

    L = 128
    P = L + 6
    R = np.zeros((P, L), np.float32)
    R[0, 2] = 2.0
    R[1, 1] = 1.5
    R[2, 0] = 1.25
    for i in range(L):
        R[3 + i, i] = 1.0
    R[P - 3, L - 1] = 1.25
    R[P - 2, L - 2] = 1.5
    R[P - 1, L - 3] = 2.0

    A = np.zeros((3, L, 256), np.float32)
    for cc in range(3):
        Me = np.zeros((P, L), np.float32)
        Mo = np.zeros((P, L), np.float32)
        for v in range(L):
            for j in range(5):
                Me[v + 5 - j, v] += inv[cc, 2 * j]
            for j in range(4):
                Mo[v + 5 - j, v] += inv[cc, 2 * j + 1]
        A[cc, :, 0:128] = R.T @ Me
        A[cc, :, 128:256] = R.T @ Mo
    return A


_CACHE = {}


def _get_nc():
    if "nc" in _CACHE:
        return _CACHE["nc"]

    import concourse.bacc as bacc
    import concourse.tile as tile
    from concourse import mybir

    f32 = mybir.dt.float32
    dt_mm = mybir.dt.bfloat16 if _USE_BF16 else mybir.dt.float32r

    nc = bacc.Bacc("TRN2", target_bir_lowering=False, debug=False, num_devices=N_CORES)
    # x arrives host-pre-transposed to [b, w, c, h] so every DMA descriptor is a
    # full 36KB-per-partition contiguous run and pass-1 weight slices are contiguous.
    x_ext = nc.declare_dram_parameter("x", [B_PER, W, C, H], dt_mm, isOutput=False)
    a_ext = nc.declare_dram_parameter("amat", [128, 3 * 256], dt_mm, isOutput=False)
    o_ext = nc.declare_dram_parameter("out", [B_PER, 2 * H, 2 * W, G2], f32, isOutput=True)

    with tile.TileContext(nc) as tc:
        with tc.tile_pool(name="const", bufs=1) as cpool, \
             tc.tile_pool(name="xp", bufs=2) as xpool, \
             tc.tile_pool(name="yp", bufs=1) as ypool, \
             tc.tile_pool(name="st", bufs=2) as spool, \
             tc.tile_pool(name="zp", bufs=4, space="PSUM") as zpool, \
             tc.tile_pool(name="op", bufs=3, space="PSUM") as opool:

            amat = cpool.tile([128, 3 * 256], dt_mm, tag="amat")
            amat_mm = amat[:]
            amat_loaded = False

            for b in range(B_PER):
                # ---- load x[b]: SBUF [w; (c, h)], contiguous; c-blocks pipeline pass 1 ----
                x_sb = xpool.tile([128, C, H], dt_mm, tag="x")
                bounds = [0, 6, 24, 48, 72, 96, 120, 144] if b == 0 else [0, 48, 96, 144]
                for c0, c1 in zip(bounds, bounds[1:]):
                    nc.sync.dma_start(
                        out=x_sb[:, c0:c1, :],
                        in_=x_ext[b, :, c0:c1, :],
                    )
                    if not amat_loaded:
                        # after the first x block so pass-1 deps clear earliest
                        nc.sync.dma_start(out=amat[:], in_=a_ext[:])
                        amat_loaded = True
                x_mm = x_sb[:]

                # ---- pass 1: z[h, g, v] ----
                y_sb = ypool.tile([128, G, 256], dt_mm, tag="y")
                for g in range(G):
                    zp = zpool.tile([128, 256], f32, tag="z")
                    for cc in range(3):
                        nc.tensor.matmul(
                            out=zp[:],
                            lhsT=x_mm[:, 3 * g + cc, :],
                            rhs=amat_mm[:, cc * 256:(cc + 1) * 256],
                            start=(cc == 0),
                            stop=(cc == 2),
                        )
                    if g % 2 == 0:
                        nc.vector.tensor_copy(y_sb[:, g, :], zp[:])
                    else:
                        nc.scalar.copy(y_sb[:, g, :], zp[:])

                # g -> (G2, gg) view for pass-2 rhs slices
                y_mm = y_sb[:].rearrange(
                    "p (gtwo gg) v -> p gg gtwo v", gg=3)

                # ---- pass 2 + interleave + store ----
                for r in range(2):  # output-row phase: h' = 2*vh + r
                    stage = spool.tile([128, 2 * W * G2], f32, tag="stage")
                    # free idx = w'*16 + c, w' = wb*64 + wv*2 + par
                    sv = stage[:].rearrange(
                        "p (wb wv par c) -> p wb par c wv", wb=4, wv=32, par=2, c=16)
                    out_view = o_ext[b].rearrange("(vh two) w c -> two vh (w c)", two=2)
                    # vblk pairs (even-w' block, odd-w' block) grouped so each
                    # w'-quarter of `stage` completes early and can stream out
                    for hw_half, vblks in enumerate(((0, 4), (1, 5), (2, 6), (3, 7))):
                        for vblk in vblks:
                            op = opool.tile([128, G2, 32], f32, tag="o2")
                            for gg in range(3):
                                nc.tensor.matmul(
                                    out=op[:],
                                    lhsT=amat_mm[:, gg * 256 + r * 128: gg * 256 + r * 128 + 128],
                                    rhs=y_mm[:, gg, :, vblk * 32:(vblk + 1) * 32],
                                    start=(gg == 0),
                                    stop=(gg == 2),
                                )
                            par, wb = (0, vblk) if vblk < 4 else (1, vblk - 4)
                            if vblk % 2 == 0:
                                nc.vector.tensor_copy(sv[:, wb, par, :, :], op[:])
                            else:
                                nc.scalar.copy(sv[:, wb, par, :, :], op[:])
                        eng = nc.gpsimd if hw_half % 2 == 0 else nc.sync
                        eng.dma_start(
                            out=out_view[r, :, hw_half * 1024:(hw_half + 1) * 1024],
                            in_=stage[:, hw_half * 1024:(hw_half + 1) * 1024])

    nc.compile()
    _CACHE["nc"] = nc
    return nc


def kernel(x: np.ndarray) -> np.ndarray:
    from concourse.bass_utils import run_bass_kernel_spmd

    assert x.shape == (B_FULL, H, W, C), x.shape
    import ml_dtypes
    dt_np = ml_dtypes.bfloat16 if _USE_BF16 else np.float32
    # [b, h, w, c] -> [b, w, c, h]: contiguous per-partition DMA runs on device
    x = np.ascontiguousarray(x.transpose(0, 2, 3, 1).astype(dt_np))
    amat = np.ascontiguousarray(
        _build_A().transpose(1, 0, 2).reshape(128, 3 * 256).astype(dt_np))

    nc = _get_nc()
    in_maps = [
        {"x": x[i * B_PER:(i + 1) * B_PER], "amat": amat} for i in range(N_CORES)
    ]
    res = run_bass_kernel_spmd(nc, in_maps, list(range(N_CORES)))
    out = np.concatenate([res.results[i]["out"] for i in range(N_CORES)], axis=0)
    return out
